# revision 18
# baseline (speedup 1.0000x reference)
"""ColBERT MaxSim kernel for 8 Trainium2 NeuronCores (Bass/Tile).

Strategy: data-parallel over the 256-doc batch (32 docs per core).

Host side:
  - compacts each doc's VALID tokens (d_mask is ~50% dense) to the
    front and pads to a per-quad budget with a COPY of the doc's first
    valid token.  Duplicating a valid token leaves the per-(query,doc)
    max unchanged, so this is exactly equivalent to -inf masking.
  - VARIABLE per-quad budgets: each core holds 4 query batches x 8
    docs; each batch's docs are sorted by valid count, and quad g takes
    the rank-g doc of every batch (position d <-> batch d, so the
    kernel's doc->query mapping stays compile-time static).  Budget
    lt[g] = max valid count over quad-slot g across all cores (rounded
    to 16), so early quads are ~288 tokens and late ones ~240.
  - computes the query side on host in fp32: qp = l2norm(W @ q)
    [dim, 128q] bf16 per core; the device contracts sim over dim=128
    off the projected docs (NOT over H=768 off the raw stream).
  - W is pre-scaled by 8 so fp8(e4m3) entries land in the normal
    range; the scale cancels exactly in sim * rsqrt(|8Wd|^2).
  - the final sum over the 32 query tokens runs on host (it is a
    [4,32,8] reduction of the DMA'd max table).

Per core (32 docs = 8 quads), DMA-bound design (~6.8 MB fp8 in):
  slab DMA triggers all sit on the (otherwise idle) sync queue.  The
  slab pool recycles 3 buffers, so the DMA for slab g+3 carries a WAR
  wait on quad g's consumption: this demand-paces the stream with 3
  slabs of lookahead.  In-flight DMAs share bandwidth FAIRLY, so
  issuing everything at once would make slab0 finish last — staggering
  is essential.
  per pair of docs (DoubleRow fp8, K=256 per pass):
    pd[:, j] = W8.T @ dT[d]            [128dim, lt] f32 PSUM
    pdb = bf16(pd)                     (DVE copy -> SBUF)
    sq  = pd^2                         (ACT square, bf16 -> SBUF)
  per quad g (4 docs, epilogue fires one pair into the next quad):
    ssq[32d:+32, :] = ones[:, :32].T @ sq[d]    (per-token sumsq)
    sim[32d:+32, :] = qp[:, b].T @ pdb[d]       (K=128 bf16)
    invb = rsqrt(ssq + eps)            (ACT)
    scaled = sim * invb                (DVE, bf16)
    maxcol[:, g] = max_tok(scaled)     (DVE reduce_max)
  maxcol [128q, 8] is DMA'd out; host sums each 32-query block.

PE warm-up: the HAM clock gate keeps the PE at 1.2 GHz until it sees a
few us of sustained FULL-ARRAY activity (tiny matmuls don't register);
13 x N=512 full-width bf16 matmuls (~3us) run while the first slabs
are in flight.  Small filler matmuls at each quad boundary keep the
PE's DMA-wait gaps under the ~250ns clock-drop threshold.
"""

import numpy as np
import ml_dtypes

import concourse.bass as bass
import concourse.bacc as bacc
import concourse.mybir as mybir
import concourse.tile as tile
from concourse.bass_utils import run_bass_kernel_spmd

N_CORES = 8
H, HC, P = 768, 6, 128   # hidden dim, h-chunks, partitions
DIM = 128                # projection dim
DPC = 32                 # docs per core
QPC = 128                # query vectors per core (4 batches x 32)
PPQ = 8                  # passages per query
NQUAD = DPC // 4
BF16 = mybir.dt.bfloat16
FP8 = mybir.dt.float8e4
F32 = mybir.dt.float32
EPS2 = 1e-12
LT_MIN = 64              # floor on per-quad compacted token budget
W8SCALE = 8.0            # fp8 pre-scale on W; cancels in normalization
N_WARMUP = 10            # full-array warmup matmuls (N=512 each)
N_FILLER = 5             # small filler matmuls per quad boundary

_LTS = (288,) * NQUAD
_NC_CACHE = {}


def _rsqrt_act(nc, out, in_, bias_ap):
    """out = 1/sqrt(in_ + bias). Emits the Rsqrt activation directly
    (bass's helper refuses it; the 40k-entry reciprocal_sqrt HW table is
    plenty accurate for this kernel's fp8-dominated error budget)."""
    eng = nc.scalar
    ins = [eng.lower_ap(in_), eng.lower_ap(bias_ap),
           mybir.ImmediateValue(dtype=mybir.dt.float32, value=1.0),
           mybir.ImmediateValue(dtype=mybir.dt.float32, value=0.0)]
    return eng.add_instruction(mybir.InstActivation(
        name=nc.get_next_instruction_name(),
        func=mybir.ActivationFunctionType.Rsqrt,
        ins=ins, outs=[eng.lower_ap(out)]))


def _build_nc(lts):
    nc = bacc.Bacc()
    dt_d = [nc.declare_dram_parameter(f"dt{g}", [P, 4, HC, lts[g]], FP8,
                                      isOutput=False) for g in range(NQUAD)]
    qp_d = nc.declare_dram_parameter("qp", [DIM, 4, 32], BF16, isOutput=False)
    wt8_d = nc.declare_dram_parameter("wt8", [P, HC, DIM], FP8,
                                      isOutput=False)
    out_d = nc.declare_dram_parameter("out", [P, NQUAD], F32, isOutput=True)
    DR = mybir.MatmulPerfMode.DoubleRow

    with tile.TileContext(nc) as tc:
        with (
            tc.tile_pool(name="const", bufs=1) as const,
            tc.tile_pool(name="slab", bufs=4) as slabp,
            tc.tile_pool(name="work", bufs=2) as work,
            tc.tile_pool(name="psum", bufs=2, space=bass.MemorySpace.PSUM) as psum,
        ):
            # ---- input DMAs: all triggers on the (otherwise idle) sync
            # queue.  The slab pool recycles 3 buffers, so the DMA for
            # slab g+3 carries a WAR wait on quad g's consumption: this
            # demand-paces the stream with 3 slabs of lookahead.
            wt8_s = const.tile([P, HC, DIM], FP8)
            nc.sync.dma_start(out=wt8_s, in_=wt8_d[:])
            slabs = {}
            for g in range(NQUAD):
                slabs[g] = slabp.tile([P, 4, HC, lts[g]], FP8,
                                      tag="slab", name=f"slab{g}")
            nc.sync.dma_start(out=slabs[0][:, 0:2], in_=dt_d[0][:, 0:2])
            nc.sync.dma_start(out=slabs[0][:, 2:4], in_=dt_d[0][:, 2:4])
            qp_s = const.tile([DIM, 4, 32], BF16)
            nc.sync.dma_start(out=qp_s, in_=qp_d[:])
            for g in range(1, NQUAD):
                nc.sync.dma_start(out=slabs[g], in_=dt_d[g][:])

            # ---- constants ----
            w_l = const.tile([P, 512], BF16)       # warmup lhsT/rhs
            nc.vector.memset(w_l, 1.0)
            ones_s = const.tile([P, 32], BF16)     # all-ones lhsT for ssq
            nc.scalar.copy(ones_s, w_l[:, :32])
            # PE warm-up on memset data (no DMA dependency): the HAM
            # clock gate meters ARRAY activity, so these are full-width
            # K=128 x M=128 x N=512 matmuls
            warm = psum.tile([P, 512], F32, tag="warm", bufs=1)
            for i in range(N_WARMUP):
                nc.tensor.matmul(warm, w_l[:, :128], w_l,
                                 start=True, stop=True)
            eps_t = const.tile([P, 1], F32)        # rsqrt bias (l2norm eps^2)
            nc.vector.memset(eps_t, EPS2)
            maxcol = const.tile([P, NQUAD], F32)   # [4batch x 32q, quads]

            state = {}

            def emit_epi(g, halves=1, only=None):
                # halves=2 processes docs {0,1} then {2,3} separately to
                # shorten the serial tail chain of the final quad
                sq4, pdb = state[g]
                lt = lts[g]
                step = 4 // halves
                for h in range(halves):
                    if only is not None and h != only:
                        continue
                    # each half gets its own PSUM bank: PE writing a bank
                    # while ACT/DVE read it is a fatal HW collision
                    ssq = psum.tile([P, 512], F32, tag="ssq")
                    sim = psum.tile([P, 512], F32, tag="sim", bufs=1)
                    r0, r1 = 32 * h * step, 32 * (h + 1) * step
                    for d in range(h * step, (h + 1) * step):
                        nc.tensor.matmul(ssq[32 * d:32 * d + 32, :lt],
                                         ones_s, sq4[:, d, :],
                                         start=True, stop=True,
                                         tile_position=(0, 32 * d))
                    for d in range(h * step, (h + 1) * step):
                        nc.tensor.matmul(sim[32 * d:32 * d + 32, :lt],
                                         qp_s[:, d, :], pdb[:, d, :],
                                         start=True, stop=True,
                                         tile_position=(0, 32 * d))
                    invb = work.tile([P, lt], F32, tag="invb")
                    _rsqrt_act(nc, invb[r0:r1], ssq[r0:r1, :lt], eps_t[r0:r1])
                    scaled = work.tile([P, lt], BF16, tag="scaled")
                    nc.vector.tensor_mul(scaled[r0:r1], sim[r0:r1, :lt],
                                         invb[r0:r1])
                    nc.vector.reduce_max(out=maxcol[r0:r1, g:g + 1],
                                         in_=scaled[r0:r1],
                                         axis=mybir.AxisListType.X)

            def emit_proj(pp):
                g, lt, slab = pp // 2, lts[pp // 2], slabs[pp // 2]
                pd = psum.tile([DIM, 2, 512], F32, tag="pd")
                for c in range(0, HC, 2):
                    for j in range(2):
                        d = 2 * (pp % 2) + j
                        nc.tensor.matmul(pd[:, j, :lt], wt8_s[:, c:c + 2, :],
                                         slab[:, d, c:c + 2, :],
                                         start=(c == 0), stop=(c == HC - 2),
                                         perf_mode=DR)
                return pd

            def emit_copies(pp, pd):
                g, lt = pp // 2, lts[pp // 2]
                sq4, pdb = state[g]
                pr = pp % 2
                nc.vector.tensor_copy(pdb[:, 2 * pr:2 * pr + 2, :],
                                      pd[:, :, :lt])
                nc.scalar.square(sq4[:, 2 * pr:2 * pr + 2, :], pd[:, :, :lt])

            # ---- doc loop: 16 pairs; quad epilogue fires one pair late ----
            for pp in range(DPC // 2):
                g = pp // 2
                lt = lts[g]
                if pp % 2 == 0:
                    sq4 = work.tile([P, 4, lt], BF16, tag="sq4")
                    pdb = work.tile([P, 4, lt], BF16, tag="pdb")
                    state[g] = (sq4, pdb)
                    if pp >= 2:
                        # fillers keep the PE's DMA-wait gap at the quad
                        # boundary below the HAM clock-drop threshold
                        for i in range(N_FILLER):
                            nc.tensor.matmul(warm[0:32, :192],
                                             w_l[:, :32], w_l[:, :192],
                                             start=True, stop=True)
                    if pp >= 4:
                        # the epilogue runs TWO pairs behind its quad: all
                        # its inputs are long since ready, so these PE ops
                        # execute immediately and fill the slab-DMA wait
                        emit_epi(pp // 2 - 2)
                if pp == DPC // 2 - 1:
                    # last pair: the previous quad's epilogue goes FIRST so
                    # its PE/DVE work happens while we wait on the final
                    # slab's DMA, then a fine-grained per-half tail.
                    emit_epi(g - 1)
                    pd = emit_proj(pp)
                    emit_copies(pp, pd)
                    emit_epi(g, halves=2, only=0)
                    emit_epi(g, halves=2, only=1)
                else:
                    pd = emit_proj(pp)
                    emit_copies(pp, pd)

            # ---- writeback (host does the 32-query sums; the scalar
            # queue is idle after the last rsqrt, so it fires promptly) ----
            nc.scalar.dma_start(out=out_d[:], in_=maxcol)
    nc.compile()
    return nc


def _get_nc():
    nc = _NC_CACHE.get(_LTS)
    if nc is None:
        nc = _NC_CACHE[_LTS] = _build_nc(_LTS)
    return nc


def _prep_in_maps(q_hidden, d_hidden, W, d_mask):
    global _LTS, _PERM
    f8 = ml_dtypes.float8_e4m3
    cnt = d_mask.sum(1)
    order = np.argsort(~d_mask, axis=1, kind="stable")
    # quad g on every core = the rank-g doc (by valid count) of each of
    # the core's 4 query batches; position within quad = batch index
    perm = np.zeros((N_CORES, NQUAD, 4), dtype=np.int64)
    for c in range(N_CORES):
        for b in range(4):
            docs = np.arange((4 * c + b) * PPQ, (4 * c + b + 1) * PPQ)
            perm[c, :, b] = docs[np.argsort(-cnt[docs], kind="stable")]
    _PERM = perm
    lts = tuple(int(max(LT_MIN, (int(cnt[perm[:, g, :]].max()) + 15)
                        // 16 * 16)) for g in range(NQUAD))
    _LTS = lts
    wt_t = np.ascontiguousarray(W.T.reshape(HC, P, DIM).transpose(1, 0, 2))
    wt8 = (wt_t * W8SCALE).astype(f8)
    # query side on host: qp = l2norm(W @ q)  [dim, 4, 32] bf16 per core
    qf = q_hidden.reshape(-1, H).astype(np.float32)          # [1024q, H]
    qp = qf @ W.T                                            # [1024q, dim]
    qp /= np.maximum(np.sqrt((qp * qp).sum(-1, keepdims=True)), 1e-12)
    qp = qp.astype(ml_dtypes.bfloat16)
    in_maps = []
    for c in range(N_CORES):
        m = {"wt8": wt8}
        for g in range(NQUAD):
            lt = lts[g]
            ids = perm[c, g, :]                               # 4 global docs
            idxg = np.where(np.arange(lt)[None, :] >= cnt[ids][:, None],
                            order[ids, :1], order[ids, :lt])
            dcg = np.take_along_axis(d_hidden[ids], idxg[:, :, None], axis=1)
            dtg = dcg.astype(f8).transpose(0, 2, 1)           # [4, 768, lt]
            dtg = dtg.reshape(4, HC, P, lt).transpose(2, 0, 1, 3)
            m[f"dt{g}"] = np.ascontiguousarray(dtg)           # [P, 4, HC, lt]
        qsl = qp[c * QPC:(c + 1) * QPC]                       # [128q, dim]
        m["qp"] = np.ascontiguousarray(qsl.T.reshape(DIM, 4, 32))
        in_maps.append(m)
    return in_maps


def _run(in_maps, trace=False, **kw):
    res = run_bass_kernel_spmd(
        _get_nc(), in_maps, core_ids=list(range(N_CORES)), trace=trace, **kw)
    # per-core output is maxcol [128, NQUAD]: rows = 4 batches x 32 query
    # tokens; host sums each 32-row block -> score [b, g] for doc
    # _PERM[core, g, b]
    out = np.zeros(N_CORES * DPC, dtype=np.float32)
    for c in range(N_CORES):
        r = res.results[c]["out"].astype(np.float32)          # [128, NQUAD]
        r = r.reshape(4, 32, NQUAD).sum(axis=1)               # [4, NQUAD]
        for g in range(NQUAD):
            out[_PERM[c, g, :]] = r[:, g]
    return out, res


def kernel(q_hidden, d_hidden, W, d_mask, ppq):
    q_hidden = np.asarray(q_hidden, dtype=np.float32)
    d_hidden = np.asarray(d_hidden, dtype=np.float32)
    W = np.asarray(W, dtype=np.float32)
    d_mask = np.asarray(d_mask).astype(bool)
    in_maps = _prep_in_maps(q_hidden, d_hidden, W, d_mask)
    out, _ = _run(in_maps, trace=False)
    return out
